# revision 39
# baseline (speedup 1.0000x reference)
"""BandSplit kernel for Trainium2 (8 NeuronCores, SPMD data-parallel over batch).

Reference computation (per band i, band width b, c=2b):
    xb[b,t,c]   = x[b, f0:f0+b, t, :] transposed/reshaped     (B, T, c)
    GroupNorm(1, c) over (T, c) per sample, affine gn_w/gn_b
    Linear(c -> 128) with fc_w/fc_b
    out stacked over 31 bands -> [B, T, 128, 31]

Algebra: per sample the op is affine in x:
    z[t,o,i] = sum_c (x[t,c]*s_i) * (gn_w[i,c]*fc_w[i,o,c])
               + beta[i,o] + (-mu_i*s_i) * g[i,o]
with s=rsqrt(var+eps), beta=fc_b+fc_w@gn_b, g=fc_w@gn_w.  The s_i scale is
applied to the data rows during the even/odd de-interleave; the static
weights are packed on host.  The per-sample bias row is contracted by the
main matmul itself: each weight tile keeps one spare contraction row whose
xg counterpart is constant 1.0 and whose weight values (the bias) are
DMA-written per sample.  PSUM drains are then plain copies.  The one tile
with no spare row (4x16 bands = 128 rows) keeps a vector add-drain fed by
a PE-broadcast bias block.

Layout: contraction rows are packed per-tile as [E rows | O rows | bias]
so the de-interleaved data moves into the matmul tiles with 2 slab DMAs
per tile (4 when a tile straddles an f-tile boundary).  Weight columns are
(o-major, band-minor) interleaved per matmul group; each group is exactly
one PSUM bank.
"""

import os
import numpy as np

import concourse.bass as bass
import concourse.tile as tile
import concourse.mybir as mybir
from concourse.bass_utils import run_bass_kernel_spmd

# ----------------------------------------------------------------------------
# Problem constants (hardcoded; kernel.py must be self-contained)
# ----------------------------------------------------------------------------
BANDS = [2, 3, 3, 3, 3, 3, 3, 3, 3, 3, 3, 8, 8, 8, 8, 8, 8, 8, 8, 8, 8, 8, 8,
         16, 16, 16, 16, 16, 16, 16, 17]
NB = len(BANDS)           # 31
CH = 128                  # output channels per band
EPS = 1e-5
B_FULL, F, T = 16, 257, 1000
N_CORES = 8
B_LOC = B_FULL // N_CORES  # 2 samples per core

F0 = [sum(BANDS[:i]) for i in range(NB)]  # band start freq

CHUNKS = [(t0, min(128, T - t0)) for t0 in range(0, T, 128)]

# f-tiles of the raw input
FT = [(0, 128), (128, 112), (240, 17)]
FT_BANDS = [(0, 23), (23, 30), (30, 31)]

# contraction tiles: (band_lo, band_hi, f_start); rows = [E | O | bias]
TILES = [(0, 8, 0), (8, 16, 23), (16, 20, 72),
         (20, 24, 104), (24, 28, 144), (28, 31, 208)]
N_XT = len(TILES)
ESUM = [sum(BANDS[lo:hi]) for (lo, hi, _) in TILES]   # E-slab rows per tile
# tile 4 (4x16 bands) has no spare row for the bias
BIAS_ROW = [2 * e if 2 * e < 128 else None for e in ESUM]
TILE_ROWS = [2 * e + (0 if BIAS_ROW[t] is None else 1)
             for t, e in enumerate(ESUM)]

# groups == banks, one matmul each: (tile, blo, bhi, bank)
GROUPS = [(0, 0, 4, 0), (0, 4, 8, 1),
          (1, 8, 12, 2), (1, 12, 16, 3),
          (2, 16, 20, 4), (3, 20, 24, 5),
          (4, 24, 28, 6), (5, 28, 31, 7)]
# wt/psum column offset of group g is bank*512; bank7 only 384 wide
GCOLS = [(bhi - blo) * CH for (_, blo, bhi, _) in GROUPS]
WTOFF = [512 * g for g in range(8)]

# contraction rows per group (tile rows incl. bias row; earlier groups of a
# shared tile contract the later bands' rows with zero weights)
GREND = [TILE_ROWS[t] for (t, _, _, _) in GROUPS]

# drain engine per group: "v" vector copy, "s" scalar copy,
# "vb" vector copy+bias-add (tile-4 group, no spare bias row)
DRAIN_ENG = ["v", "v", "v", "v", "s", "s", "vb", "s"]

# slab DMA plan: tile -> list of (f_tile, src_row, dst_row, rows)
SLABS = []
for t, (lo, hi, fst) in enumerate(TILES):
    fend = fst + ESUM[t]
    segs = []
    for g, (f0g, P) in enumerate(FT):
        a, b = max(fst, f0g), min(fend, f0g + P)
        if a < b:
            segs.append((g, a - f0g, a - fst, b - a))
    SLABS.append(segs)


def _build_const_tables(gn_w, gn_b, fc_w, fc_b):
    """Host-side packing of the (tiny) parameters into matmul-ready tables."""
    f16 = np.float16
    # wt_all [128, 3968]: group-major cols, (o, band)-interleaved per group
    wt = np.zeros((128, CH * NB), np.float32)
    for gi, (t, blo, bhi, bank) in enumerate(GROUPS):
        _, _, fst = TILES[t]
        nb = bhi - blo
        for il, i in enumerate(range(blo, bhi)):
            b = BANDS[i]
            w = fc_w[i].astype(np.float64)                 # [128, 34]
            for k in range(b):
                floc = F0[i] + k - fst
                cE, cO = 2 * k, 2 * k + 1
                col = WTOFF[gi] + np.arange(CH) * nb + il
                wt[floc, col] = gn_w[i, cE] * w[:, cE]
                wt[ESUM[t] + floc, col] = gn_w[i, cO] * w[:, cO]

    # mbias [63, 4096]: same column order; row 62 -> beta, row 31+i -> g
    mbias = np.zeros((63, 4096), np.float32)
    for gi, (t, blo, bhi, bank) in enumerate(GROUPS):
        nb = bhi - blo
        for il, i in enumerate(range(blo, bhi)):
            c = 2 * BANDS[i]
            w = fc_w[i, :, :c].astype(np.float64)          # [128, c]
            beta = fc_b[i] + w @ gn_b[i, :c]               # [128]
            gvec = w @ gn_w[i, :c]                         # [128]
            cols = bank * 512 + np.arange(CH) * nb + il
            mbias[62, cols] = beta
            mbias[31 + i, cols] = gvec

    # band indicator, transposed [31, 257] (the [P,31] form is produced
    # on-device via a PE transpose to keep DMA packet counts low)
    ind = np.zeros((F, NB), np.float32)
    for i in range(NB):
        ind[F0[i]:F0[i] + BANDS[i], i] = 1.0
    indT = np.ascontiguousarray(ind.T).astype(f16)  # [31, 257]

    invct = np.array([1.0 / (2 * b * T) for b in BANDS], np.float32)
    invct2 = np.concatenate([invct, invct])[None, :]

    ind128 = np.zeros((128, 3 * NB), np.float32)
    for g, (f0g, P) in enumerate(FT):
        ind128[:P, 31 * g:31 * g + 31] = ind[f0g:f0g + P, :]
    ones1000 = np.ones((1, T), f16)
    return wt.astype(f16), mbias.astype(f16), indT, invct2, ind128, ones1000


# ----------------------------------------------------------------------------
# Bass kernel
# ----------------------------------------------------------------------------
_NC_CACHE = {}


def _spill_waits(nc):
    """Split multi-wait instructions into NoOp(wait) + instruction.

    The walrus build in this container enforces the HW wait capacity
    (1 sync wait per instruction, 2 for EventSemaphore); Tile emits more.
    Engine queues are in-order, so hoisting extra waits into preceding
    NoOps on the same queue preserves semantics.
    """
    n = 0
    for fn in nc.m.functions:
        for bb in fn.blocks:
            out = []
            changed = False
            for inst in bb.instructions:
                si = getattr(inst, "sync_info", None)
                cap = 2 if isinstance(inst, mybir.InstEventSemaphore) else 1
                if si is not None and si.on_wait and len(si.on_wait) > cap:
                    waits = list(si.on_wait)
                    extra, keep = waits[:-cap], waits[-cap:]
                    for w in extra:
                        nop = mybir.InstNoOp(name=f"{inst.name}_w{n}",
                                             ins=[], outs=[])
                        nop.engine = inst.engine
                        nop.sync_info = mybir.SyncInfo(on_wait=[w],
                                                       on_update=[])
                        out.append(nop)
                        n += 1
                    si.on_wait = keep
                    changed = True
                out.append(inst)
            if changed:
                bb.instructions = out
    return n


def build_bass():
    repeat = int(os.environ.get("BS_REPEAT", "1"))
    key = (repeat,)
    if key in _NC_CACHE:
        return _NC_CACHE[key]
    F32 = mybir.dt.float32
    F16 = mybir.dt.float16

    nc = bass.Bass("TRN2", target_bir_lowering=False, debug=False,
                   num_devices=N_CORES)

    x_d = nc.dram_tensor("x", [B_LOC, F, T, 2], F32, kind="ExternalInput").ap()
    wt_d = nc.dram_tensor("wt", [128, CH * NB], F16,
                          kind="ExternalInput").ap()
    mbias_d = nc.dram_tensor("mbias", [63, 4096], F16,
                             kind="ExternalInput").ap()
    indT_d = nc.dram_tensor("indT", [NB, F], F16, kind="ExternalInput").ap()
    ind128_d = nc.dram_tensor("ind128", [128, 3 * NB], F32,
                              kind="ExternalInput").ap()
    ones_d = nc.dram_tensor("ones1000", [1, T], F16,
                            kind="ExternalInput").ap()
    invct_d = nc.dram_tensor("invct2", [1, 2 * NB], F32,
                             kind="ExternalInput").ap()
    z_d = nc.dram_tensor("z", [B_LOC, T, CH * NB], F16,
                         kind="ExternalOutput").ap()

    AluOp = mybir.AluOpType
    ActFn = mybir.ActivationFunctionType

    with tile.TileContext(nc) as tc:
        with (
            tc.tile_pool(name="const", bufs=1) as constp,
            tc.tile_pool(name="a", bufs=6) as ap_,
            tc.tile_pool(name="sq", bufs=2) as sqp,
            tc.tile_pool(name="eo", bufs=4) as eop,
            tc.tile_pool(name="xg", bufs=1) as xgp,
            tc.tile_pool(name="small", bufs=6) as smp,
            tc.tile_pool(name="per", bufs=1) as perp,
            tc.tile_pool(name="out", bufs=3) as outp,
            tc.tile_pool(name="psum", bufs=1, space="PSUM") as psp,
        ):
            QS = [nc.sync, nc.gpsimd]

            # const tiles (DMAs are emitted after the A-loads; see below)
            wt_all = constp.tile([128, CH * NB], F16, tag="wt")
            mbias_sb = constp.tile([63, 4096], F16, tag="mbias")
            indT_sb = [constp.tile([NB, P], F16, tag=f"indT_{g}",
                                   name=f"indT_{g}")
                       for g, (f0, P) in enumerate(FT)]
            ind128 = constp.tile([128, 3 * NB], F32, tag="ind128")
            ind_sb = [ind128[0:P, 31 * g:31 * g + 31]
                      for g, (f0, P) in enumerate(FT)]
            invct_sb = constp.tile([1, 2 * NB], F32, tag="invct")
            ident = constp.tile([1, 1], F32, tag="ident")
            epsc = constp.tile([1, 1], F32, tag="epsc")
            ones63 = constp.tile([63, 128], F16, tag="ones63")
            ind2p = constp.tile([68, NB], F32, tag="ind2p")
            ones_row = constp.tile([1, T], F16, tag="ones_row")
            # (ones_row loaded from dram, 1 packet)

            def emit_consts():
                for g, (f0, P) in enumerate(FT):
                    nc.sync.dma_start(indT_sb[g][:], indT_d[:, f0:f0 + P])
                nc.sync.dma_start(invct_sb[:], invct_d[:])
                nc.sync.dma_start(ones_row[:], ones_d[:])
                nc.sync.dma_start(ind128[:], ind128_d[:])
                nc.sync.dma_start(mbias_sb[:], mbias_d[:])
                nc.gpsimd.dma_start(wt_all[:], wt_d[:])
                nc.vector.memset(ident[:], 1.0)
                nc.vector.memset(epsc[:], EPS)
                nc.vector.memset(ones63[:], 1.0)
                nc.vector.memset(ind2p[:], 0.0)
                nc.vector.memset(ind2p[:, 30:31], 1.0)

            # persistent per-sample tiles
            xg = [[xgp.tile([TILE_ROWS[t], T], F16, tag=f"xg_{s}_{t}",
                            name=f"xg_{s}_{t}")
                   for t in range(N_XT)] for s in range(B_LOC)]
            v63 = [perp.tile([63, 1], F16, tag=f"v63_{s}", name=f"v63_{s}")
                   for s in range(B_LOC)]
            sfrow = [[perp.tile([P, 1], F32, tag=f"sf_{s}_{g}",
                                name=f"sf_{s}_{g}")
                      for g, (f0, P) in enumerate(FT)] for s in range(B_LOC)]
            # bias row (for the wt spare-row writes) and the tile-4 block
            brow = [perp.tile([1, 4096], F16, tag=f"brow_{s}",
                              name=f"brow_{s}") for s in range(B_LOC)]
            bblk = [perp.tile([128, 512], F16, tag=f"bblk_{s}",
                              name=f"bblk_{s}") for s in range(B_LOC)]
            v63rep = [perp.tile([63, 128], F16, tag=f"v63r_{s}",
                                name=f"v63r_{s}") for s in range(B_LOC)]
            v63f = [perp.tile([63, 1], F32, tag=f"v63f_{s}",
                              name=f"v63f_{s}") for s in range(B_LOC)]

            def emit_setup():
                # constant-1.0 bias rows, written by 1-packet DMAs
                nc.vector.memset(ones_row[:], 1.0)
                for s in range(B_LOC):
                    for t in range(N_XT):
                        if BIAS_ROW[t] is not None:
                            QS[t % 2].dma_start(
                                xg[s][t][BIAS_ROW[t]:BIAS_ROW[t] + 1, :],
                                ones_row[:])

            pstate = {}
            aload = {}

            def emit_loads(s):
                  As = [None, None, None, None]
                  aload[s] = As
                  for g, (f0, P) in [(2, FT[2]), (0, FT[0]), (1, FT[1])]:
                      A = ap_.tile([P, 2000], F32, tag="a")
                      # split big loads across both queues to cut latency
                      h = P // 2
                      if h >= 32:
                          QS[g % 2].dma_start(
                              A[0:h, :], x_d[s, f0:f0 + h].rearrange(
                                  "p a b -> p (a b)"))
                          QS[(g + 1) % 2].dma_start(
                              A[h:P, :], x_d[s, f0 + h:f0 + P].rearrange(
                                  "p a b -> p (a b)"))
                      else:
                          # tiny f-tile: also load a packed [68,500] view so
                          # its stats cost ~1/4 of the free-size-bound ops
                          Ap = ap_.tile([4 * P, 500], F32, tag="a2p",
                                        name="A2p")
                          QS[0].dma_start(
                              A[:], x_d[s, f0:f0 + P].rearrange(
                                  "p a b -> p (a b)"))
                          QS[1].dma_start(
                              Ap[:], x_d[s, f0:f0 + P].rearrange(
                                  "p (k c) b -> (p k) (c b)", k=4))
                          As[2] = Ap
                      As[3 if g == 2 else g] = A

            def emit_stats(s):
                  mom = psp.tile([1, 2 * NB], F32, tag="bank6", name="mom")
                  stats = []
                  As = aload[s]
                  pstate[s] = (mom, As)
                  for g, (f0, P) in enumerate(FT):
                      if g == 2:
                          A = As[2]       # packed [68, 500] view (A2p)
                          P = 4 * P
                          rhs31 = ind2p
                      else:
                          A = As[g]
                          rhs31 = ind_sb[g]
                      stat = smp.tile([P, 2], F32, tag="stat")
                      stats.append(stat)
                      cols = 500 if g == 2 else 2000
                      Asq = sqp.tile([P, cols], F32, tag="sq",
                                     name="Asq")
                      nc.scalar.activation(Asq[:], A[:], ActFn.Square,
                                           accum_out=stat[:, 1:2])
                      nc.vector.tensor_reduce(stat[:, 0:1], A[:],
                                              mybir.AxisListType.X,
                                              AluOp.add)
                      b0, b1 = FT_BANDS[g]
                      nc.tensor.matmul(mom[0:1, b0:b1], lhsT=stat[:, 0:1],
                                       rhs=rhs31[:, b0:b1],
                                       start=True, stop=True)
                      nc.tensor.matmul(mom[0:1, NB + b0:NB + b1],
                                       lhsT=stat[:, 1:2],
                                       rhs=rhs31[:, b0:b1],
                                       start=True, stop=True)

            def emit_post(s):
                  mom, As = pstate[s]
                  # moments -> s, -mu*s on partition 0
                  m2 = smp.tile([1, 2 * NB], F32, tag="m2")
                  nc.vector.tensor_tensor(m2[:], mom[:], invct_sb[:],
                                          AluOp.mult)   # [mu | ex2]
                  mu = m2[:, 0:NB]
                  var = smp.tile([1, NB], F32, tag="var")
                  nc.vector.tensor_tensor(var[:], mu, mu, AluOp.mult)
                  nc.vector.tensor_tensor(var[:], m2[:, NB:2 * NB], var[:],
                                          AluOp.subtract)
                  sd = smp.tile([1, NB], F32, tag="sd")
                  nc.scalar.activation(sd[:], var[:], ActFn.Sqrt,
                                       bias=epsc[:])
                  vrow = smp.tile([1, 64], F32, tag="vrow")
                  nc.vector.reciprocal(vrow[:, 0:NB], sd[:])         # s
                  tmp = smp.tile([1, NB], F32, tag="tmp")
                  nc.vector.tensor_tensor(tmp[:], mu, vrow[:, 0:NB],
                                          AluOp.mult)
                  nc.vector.tensor_scalar(vrow[:, NB:2 * NB], tmp[:], -1.0,
                                          None, AluOp.mult)          # -mu*s
                  nc.vector.memset(vrow[:, 62:63], 1.0)

                  v63p = psp.tile([63, 1], F32, tag="bank7", name="v63p")
                  nc.tensor.transpose(v63p[:], vrow[:, 0:63], ident[:])
                  nc.vector.tensor_copy(v63[s][:], v63p[:])
                  nc.vector.tensor_copy(v63f[s][:], v63p[:])

                  # per-f-row s scale (s_frow = indT^T @ s)
                  for g, (f0, P) in enumerate(FT):
                      sfp = psp.tile([P, 1], F32, tag=f"bank{6 + g % 2}",
                                     name=f"sfp{g}")
                      nc.tensor.matmul(sfp[:], lhsT=indT_sb[g][:],
                                       rhs=v63[s][0:NB, :],
                                       start=True, stop=True)
                      nc.vector.tensor_copy(sfrow[s][g][:], sfp[:])

                  # bias values (bank-order cols): row matmuls for the spare-
                  # row tiles, a full [128,512] block for group 6 (tile 4)
                  nc.vector.tensor_scalar(v63rep[s][:], ones63[:],
                                          v63f[s][:, 0:1], None, AluOp.mult)
                  for j in range(8):
                      if j == 6:
                          bps = psp.tile([128, 512], F32, tag="bank6",
                                         name=f"bps{j}")
                          nc.tensor.matmul(
                              bps[:], lhsT=v63rep[s][:],
                              rhs=mbias_sb[:, j * 512:(j + 1) * 512],
                              start=True, stop=True)
                          nc.vector.tensor_copy(bblk[s][:], bps[:])
                      else:
                          bps = psp.tile([1, 512], F32, tag=f"bank{j % 2 + 6}",
                                         name=f"bps{j}")
                          nc.tensor.matmul(
                              bps[:], lhsT=v63[s][:],
                              rhs=mbias_sb[:, j * 512:(j + 1) * 512],
                              start=True, stop=True)
                          dst = brow[s][0:1, j * 512:(j + 1) * 512]
                          if j % 2 == 0:
                              nc.vector.tensor_copy(dst, bps[:])
                          else:
                              nc.scalar.copy(dst, bps[:])

                  # write the per-sample bias rows into wt_all spare rows
                  for t in range(N_XT):
                      if BIAS_ROW[t] is None:
                          continue
                      glist = [g for g in range(8) if GROUPS[g][0] == t]
                      c0 = WTOFF[glist[0]]
                      c1 = WTOFF[glist[-1]] + GCOLS[glist[-1]]
                      QS[t % 2].dma_start(
                          wt_all[TILE_ROWS[t] - 1:TILE_ROWS[t], c0:c1],
                          brow[s][0:1, c0:c1])

                  # de-interleave with s scaling, then slab DMAs
                  for g, (f0, P) in enumerate(FT):
                      Anat = As[g] if g < 2 else As[3]
                      Av = Anat[:].rearrange("p (t r) -> p r t", r=2)
                      E = eop.tile([P, T], F16, tag="eo", name="E")
                      O = eop.tile([P, T], F16, tag="eo", name="O")
                      nc.vector.tensor_scalar(E[:], Av[:, 0, :],
                                              sfrow[s][g][:, 0:1], None,
                                              AluOp.mult)
                      nc.scalar.activation(O[:], Av[:, 1, :], ActFn.Copy,
                                           scale=sfrow[s][g][:, 0:1])
                      for t in range(N_XT):
                          e = ESUM[t]
                          for (gg, srow, drow, rows) in SLABS[t]:
                              if gg != g:
                                  continue
                              q1 = QS[t % 2]
                              q2 = QS[(t + 1) % 2]
                              q1.dma_start(xg[s][t][drow:drow + rows, :],
                                           E[srow:srow + rows, :])
                              q2.dma_start(
                                  xg[s][t][e + drow:e + drow + rows, :],
                                  O[srow:srow + rows, :])

            # ---------------- main loop ----------------
            def emit_chunk(s, t0, M, ci=0):
                      ob = outp.tile([128, CH * NB], F16, tag="ob")
                      ob_v = ob[0:M].rearrange("p (o i) -> p o i", o=CH, i=NB)
                      pview = {j: psp.tile([128, 512], F32, tag=f"bank{j}",
                                           name=f"ps{j}") for j in range(8)}
                      for gi, (t, blo, bhi, bank) in enumerate(GROUPS):
                          n = GCOLS[gi]
                          rend = GREND[gi]
                          nc.tensor.matmul(
                              pview[bank][0:M, 0:n],
                              lhsT=xg[s][t][0:rend, t0:t0 + M],
                              rhs=wt_all[0:rend, WTOFF[gi]:WTOFF[gi] + n],
                              start=True, stop=True)
                      # drains
                      for gi, (t, blo, bhi, bank) in enumerate(GROUPS):
                          nb = bhi - blo
                          n = GCOLS[gi]
                          dst = ob_v[:, :, blo:bhi]
                          src = pview[bank][0:M, 0:n].rearrange(
                              "p (o i) -> p o i", o=CH, i=nb)
                          eng = DRAIN_ENG[gi]
                          if eng == "s":
                              nc.scalar.copy(dst, src)
                          elif eng == "v":
                              nc.vector.tensor_copy(dst, src)
                          else:  # "vb": vector copy + bias add (tile 4)
                              bia = bblk[s][0:M, 0:n].rearrange(
                                  "p (o i) -> p o i", o=CH, i=nb)
                              nc.vector.tensor_tensor(dst, src, bia,
                                                      AluOp.add)
                      zq = [nc.sync, nc.gpsimd, nc.scalar][ci % 3]
                      zq.dma_start(z_d[s, t0:t0 + M], ob[0:M, :])

            # Emission order matters: engine queues are in-order.  A-loads
            # go first (consts behind them), s1's prologue PE ops slot
            # between early s0 chunks.
            for _rep in range(repeat):
                emit_loads(0)
                if _rep == 0:
                    emit_consts()
                    emit_setup()
                emit_stats(0)
                emit_post(0)
                emit_loads(1)
                ci = 0
                for (t0, M) in CHUNKS[:3]:
                    emit_chunk(0, t0, M, ci); ci += 1
                emit_stats(1)
                for (t0, M) in CHUNKS[3:5]:
                    emit_chunk(0, t0, M, ci); ci += 1
                emit_post(1)
                for (t0, M) in CHUNKS[5:]:
                    emit_chunk(0, t0, M, ci); ci += 1
                for (t0, M) in CHUNKS:
                    emit_chunk(1, t0, M, ci); ci += 1

    _NC_CACHE[key] = nc
    return nc


# ----------------------------------------------------------------------------
# Public entry point
# ----------------------------------------------------------------------------
def kernel(x, gn_w, gn_b, fc_w, fc_b):
    x = np.asarray(x, np.float32)
    gn_w = np.asarray(gn_w, np.float32)
    gn_b = np.asarray(gn_b, np.float32)
    fc_w = np.asarray(fc_w, np.float32)
    fc_b = np.asarray(fc_b, np.float32)

    (wt_all, mbias, indT, invct2, ind128,
     ones1000) = _build_const_tables(gn_w, gn_b, fc_w, fc_b)
    nc = build_bass()
    if not getattr(nc, "_waits_spilled", False):
        _spill_waits(nc)
        nc._waits_spilled = True

    in_maps = []
    for k in range(N_CORES):
        m = {"x": np.ascontiguousarray(x[k * B_LOC:(k + 1) * B_LOC]),
             "wt": wt_all, "mbias": mbias, "indT": indT, "invct2": invct2,
             "ind128": ind128, "ones1000": ones1000}
        in_maps.append(m)
    res = run_bass_kernel_spmd(nc, in_maps, core_ids=list(range(N_CORES)))
    z16 = np.concatenate([r["z"] for r in res.results], axis=0)
    return z16.reshape(B_FULL, T, CH, NB).astype(np.float32)


# revision 40
# speedup vs baseline: 1.0582x; 1.0582x over previous
"""BandSplit kernel for Trainium2 (8 NeuronCores, SPMD data-parallel over batch).

Reference computation (per band i, band width b, c=2b):
    xb[b,t,c]   = x[b, f0:f0+b, t, :] transposed/reshaped     (B, T, c)
    GroupNorm(1, c) over (T, c) per sample, affine gn_w/gn_b
    Linear(c -> 128) with fc_w/fc_b
    out stacked over 31 bands -> [B, T, 128, 31]

Algebra: per sample the op is affine in x:
    z[t,o,i] = sum_c (x[t,c]*s_i) * (gn_w[i,c]*fc_w[i,o,c])
               + beta[i,o] + (-mu_i*s_i) * g[i,o]
with s=rsqrt(var+eps), beta=fc_b+fc_w@gn_b, g=fc_w@gn_w.  The s_i scale is
applied to the data rows during the even/odd de-interleave; the static
weights are packed on host.  The per-sample bias row is contracted by the
main matmul itself: each weight tile keeps one spare contraction row whose
xg counterpart is constant 1.0 and whose weight values (the bias) are
DMA-written per sample.  PSUM drains are then plain copies.  The one tile
with no spare row (4x16 bands = 128 rows) keeps a vector add-drain fed by
a PE-broadcast bias block.

Layout: contraction rows are packed per-tile as [E rows | O rows | bias]
so the de-interleaved data moves into the matmul tiles with 2 slab DMAs
per tile (4 when a tile straddles an f-tile boundary).  Weight columns are
(o-major, band-minor) interleaved per matmul group; each group is exactly
one PSUM bank.
"""

import os
import numpy as np

import concourse.bass as bass
import concourse.tile as tile
import concourse.mybir as mybir
from concourse.bass_utils import run_bass_kernel_spmd

# ----------------------------------------------------------------------------
# Problem constants (hardcoded; kernel.py must be self-contained)
# ----------------------------------------------------------------------------
BANDS = [2, 3, 3, 3, 3, 3, 3, 3, 3, 3, 3, 8, 8, 8, 8, 8, 8, 8, 8, 8, 8, 8, 8,
         16, 16, 16, 16, 16, 16, 16, 17]
NB = len(BANDS)           # 31
CH = 128                  # output channels per band
EPS = 1e-5
B_FULL, F, T = 16, 257, 1000
N_CORES = 8
B_LOC = B_FULL // N_CORES  # 2 samples per core

F0 = [sum(BANDS[:i]) for i in range(NB)]  # band start freq

CHUNKS = [(t0, min(128, T - t0)) for t0 in range(0, T, 128)]

# f-tiles of the raw input
FT = [(0, 128), (128, 112), (240, 17)]
FT_BANDS = [(0, 23), (23, 30), (30, 31)]

# contraction tiles: (band_lo, band_hi, f_start); rows = [E | O | bias]
TILES = [(0, 8, 0), (8, 16, 23), (16, 20, 72),
         (20, 24, 104), (24, 28, 144), (28, 31, 208)]
N_XT = len(TILES)
ESUM = [sum(BANDS[lo:hi]) for (lo, hi, _) in TILES]   # E-slab rows per tile
# tile 4 (4x16 bands) has no spare row for the bias
BIAS_ROW = [2 * e if 2 * e < 128 else None for e in ESUM]
TILE_ROWS = [2 * e + (0 if BIAS_ROW[t] is None else 1)
             for t, e in enumerate(ESUM)]

# groups == banks, one matmul each: (tile, blo, bhi, bank)
GROUPS = [(0, 0, 4, 0), (0, 4, 8, 1),
          (1, 8, 12, 2), (1, 12, 16, 3),
          (2, 16, 20, 4), (3, 20, 24, 5),
          (4, 24, 28, 6), (5, 28, 31, 7)]
# wt/psum column offset of group g is bank*512; bank7 only 384 wide
GCOLS = [(bhi - blo) * CH for (_, blo, bhi, _) in GROUPS]
WTOFF = [512 * g for g in range(8)]

# contraction rows per group (tile rows incl. bias row; earlier groups of a
# shared tile contract the later bands' rows with zero weights)
GREND = [TILE_ROWS[t] for (t, _, _, _) in GROUPS]

# drain engine per group: "v" vector copy, "s" scalar copy,
# "vb" vector copy+bias-add (tile-4 group, no spare bias row)
DRAIN_ENG = ["v", "v", "v", "v", "s", "s", "vb", "s"]

# slab DMA plan: tile -> list of (f_tile, src_row, dst_row, rows)
SLABS = []
for t, (lo, hi, fst) in enumerate(TILES):
    fend = fst + ESUM[t]
    segs = []
    for g, (f0g, P) in enumerate(FT):
        a, b = max(fst, f0g), min(fend, f0g + P)
        if a < b:
            segs.append((g, a - f0g, a - fst, b - a))
    SLABS.append(segs)


def _build_const_tables(gn_w, gn_b, fc_w, fc_b):
    """Host-side packing of the (tiny) parameters into matmul-ready tables."""
    f16 = np.float16
    # wt_all [128, 3968]: group-major cols, (o, band)-interleaved per group
    wt = np.zeros((128, CH * NB), np.float32)
    for gi, (t, blo, bhi, bank) in enumerate(GROUPS):
        _, _, fst = TILES[t]
        nb = bhi - blo
        for il, i in enumerate(range(blo, bhi)):
            b = BANDS[i]
            w = fc_w[i].astype(np.float64)                 # [128, 34]
            for k in range(b):
                floc = F0[i] + k - fst
                cE, cO = 2 * k, 2 * k + 1
                col = WTOFF[gi] + np.arange(CH) * nb + il
                wt[floc, col] = gn_w[i, cE] * w[:, cE]
                wt[ESUM[t] + floc, col] = gn_w[i, cO] * w[:, cO]

    # mbias [63, 4096]: same column order; row 62 -> beta, row 31+i -> g
    mbias = np.zeros((63, 4096), np.float32)
    for gi, (t, blo, bhi, bank) in enumerate(GROUPS):
        nb = bhi - blo
        for il, i in enumerate(range(blo, bhi)):
            c = 2 * BANDS[i]
            w = fc_w[i, :, :c].astype(np.float64)          # [128, c]
            beta = fc_b[i] + w @ gn_b[i, :c]               # [128]
            gvec = w @ gn_w[i, :c]                         # [128]
            cols = bank * 512 + np.arange(CH) * nb + il
            mbias[62, cols] = beta
            mbias[31 + i, cols] = gvec

    # band indicator, transposed [31, 257] (the [P,31] form is produced
    # on-device via a PE transpose to keep DMA packet counts low)
    ind = np.zeros((F, NB), np.float32)
    for i in range(NB):
        ind[F0[i]:F0[i] + BANDS[i], i] = 1.0
    indT = np.ascontiguousarray(ind.T).astype(f16)  # [31, 257]

    invct = np.array([1.0 / (2 * b * T) for b in BANDS], np.float32)
    invct2 = np.concatenate([invct, invct])[None, :]

    ind128 = np.zeros((128, 3 * NB), np.float32)
    for g, (f0g, P) in enumerate(FT):
        ind128[:P, 31 * g:31 * g + 31] = ind[f0g:f0g + P, :]
    ones1000 = np.ones((1, T), f16)
    return wt.astype(f16), mbias.astype(f16), indT, invct2, ind128, ones1000


# ----------------------------------------------------------------------------
# Bass kernel
# ----------------------------------------------------------------------------
_NC_CACHE = {}


def _spill_waits(nc):
    """Split multi-wait instructions into NoOp(wait) + instruction.

    The walrus build in this container enforces the HW wait capacity
    (1 sync wait per instruction, 2 for EventSemaphore); Tile emits more.
    Engine queues are in-order, so hoisting extra waits into preceding
    NoOps on the same queue preserves semantics.
    """
    n = 0
    for fn in nc.m.functions:
        for bb in fn.blocks:
            out = []
            changed = False
            for inst in bb.instructions:
                si = getattr(inst, "sync_info", None)
                cap = 2 if isinstance(inst, mybir.InstEventSemaphore) else 1
                if si is not None and si.on_wait and len(si.on_wait) > cap:
                    waits = list(si.on_wait)
                    extra, keep = waits[:-cap], waits[-cap:]
                    for w in extra:
                        nop = mybir.InstNoOp(name=f"{inst.name}_w{n}",
                                             ins=[], outs=[])
                        nop.engine = inst.engine
                        nop.sync_info = mybir.SyncInfo(on_wait=[w],
                                                       on_update=[])
                        out.append(nop)
                        n += 1
                    si.on_wait = keep
                    changed = True
                out.append(inst)
            if changed:
                bb.instructions = out
    return n


def build_bass():
    repeat = int(os.environ.get("BS_REPEAT", "1"))
    key = (repeat,)
    if key in _NC_CACHE:
        return _NC_CACHE[key]
    F32 = mybir.dt.float32
    F16 = mybir.dt.float16

    nc = bass.Bass("TRN2", target_bir_lowering=False, debug=False,
                   num_devices=N_CORES)

    x_d = nc.dram_tensor("x", [B_LOC, F, T, 2], F32, kind="ExternalInput").ap()
    wt_d = nc.dram_tensor("wt", [128, CH * NB], F16,
                          kind="ExternalInput").ap()
    mbias_d = nc.dram_tensor("mbias", [63, 4096], F16,
                             kind="ExternalInput").ap()
    indT_d = nc.dram_tensor("indT", [NB, F], F16, kind="ExternalInput").ap()
    ind128_d = nc.dram_tensor("ind128", [128, 3 * NB], F32,
                              kind="ExternalInput").ap()
    ones_d = nc.dram_tensor("ones1000", [1, T], F16,
                            kind="ExternalInput").ap()
    invct_d = nc.dram_tensor("invct2", [1, 2 * NB], F32,
                             kind="ExternalInput").ap()
    z_d = nc.dram_tensor("z", [B_LOC, T, CH * NB], F16,
                         kind="ExternalOutput").ap()

    AluOp = mybir.AluOpType
    ActFn = mybir.ActivationFunctionType

    with tile.TileContext(nc) as tc:
        with (
            tc.tile_pool(name="const", bufs=1) as constp,
            tc.tile_pool(name="a", bufs=6) as ap_,
            tc.tile_pool(name="sq", bufs=2) as sqp,
            tc.tile_pool(name="eo", bufs=4) as eop,
            tc.tile_pool(name="xg", bufs=1) as xgp,
            tc.tile_pool(name="small", bufs=6) as smp,
            tc.tile_pool(name="per", bufs=1) as perp,
            tc.tile_pool(name="out", bufs=3) as outp,
            tc.tile_pool(name="psum", bufs=1, space="PSUM") as psp,
        ):
            QS = [nc.sync, nc.gpsimd]

            # const tiles (DMAs are emitted after the A-loads; see below)
            wt_all = constp.tile([128, CH * NB], F16, tag="wt")
            mbias_sb = constp.tile([63, 4096], F16, tag="mbias")
            indT_sb = [constp.tile([NB, P], F16, tag=f"indT_{g}",
                                   name=f"indT_{g}")
                       for g, (f0, P) in enumerate(FT)]
            ind128 = constp.tile([128, 3 * NB], F32, tag="ind128")
            ind_sb = [ind128[0:P, 31 * g:31 * g + 31]
                      for g, (f0, P) in enumerate(FT)]
            invct_sb = constp.tile([1, 2 * NB], F32, tag="invct")
            ident = constp.tile([1, 1], F32, tag="ident")
            epsc = constp.tile([1, 1], F32, tag="epsc")
            ones63 = constp.tile([63, 128], F16, tag="ones63")
            ind2p = constp.tile([68, NB], F32, tag="ind2p")
            ones_row = constp.tile([1, T], F16, tag="ones_row")
            # (ones_row loaded from dram, 1 packet)

            def emit_consts():
                for g, (f0, P) in enumerate(FT):
                    nc.sync.dma_start(indT_sb[g][:], indT_d[:, f0:f0 + P])
                nc.sync.dma_start(invct_sb[:], invct_d[:])
                nc.sync.dma_start(ones_row[:], ones_d[:])
                nc.sync.dma_start(ind128[:], ind128_d[:])
                nc.sync.dma_start(mbias_sb[:], mbias_d[:])
                nc.gpsimd.dma_start(wt_all[:], wt_d[:])
                nc.vector.memset(ident[:], 1.0)
                nc.vector.memset(epsc[:], EPS)
                nc.vector.memset(ones63[:], 1.0)
                nc.vector.memset(ind2p[:], 0.0)
                nc.vector.memset(ind2p[:, 30:31], 1.0)

            # persistent per-sample tiles
            xg = [[xgp.tile([TILE_ROWS[t], T], F16, tag=f"xg_{s}_{t}",
                            name=f"xg_{s}_{t}")
                   for t in range(N_XT)] for s in range(B_LOC)]
            v63 = [perp.tile([63, 1], F16, tag=f"v63_{s}", name=f"v63_{s}")
                   for s in range(B_LOC)]
            sfrow = [[perp.tile([P, 1], F32, tag=f"sf_{s}_{g}",
                                name=f"sf_{s}_{g}")
                      for g, (f0, P) in enumerate(FT)] for s in range(B_LOC)]
            # bias row (for the wt spare-row writes) and the tile-4 block
            brow = [perp.tile([1, 4096], F16, tag=f"brow_{s}",
                              name=f"brow_{s}") for s in range(B_LOC)]
            bblk = [perp.tile([128, 512], F16, tag=f"bblk_{s}",
                              name=f"bblk_{s}") for s in range(B_LOC)]
            v63rep = [perp.tile([63, 128], F16, tag=f"v63r_{s}",
                                name=f"v63r_{s}") for s in range(B_LOC)]
            v63f = [perp.tile([63, 1], F32, tag=f"v63f_{s}",
                              name=f"v63f_{s}") for s in range(B_LOC)]

            def emit_setup():
                # constant-1.0 bias rows, written by 1-packet DMAs
                nc.vector.memset(ones_row[:], 1.0)
                for s in range(B_LOC):
                    for t in range(N_XT):
                        if BIAS_ROW[t] is not None:
                            QS[t % 2].dma_start(
                                xg[s][t][BIAS_ROW[t]:BIAS_ROW[t] + 1, :],
                                ones_row[:])

            pstate = {}
            aload = {}

            def emit_loads(s):
                  As = [None, None, None, None]
                  aload[s] = As
                  for g, (f0, P) in enumerate(FT):
                      A = ap_.tile([P, 2000], F32, tag="a")
                      # split big loads across both queues to cut latency
                      h = P // 2
                      if h >= 32:
                          QS[0].dma_start(
                              A[0:h, :], x_d[s, f0:f0 + h].rearrange(
                                  "p a b -> p (a b)"))
                          QS[1].dma_start(
                              A[h:P, :], x_d[s, f0 + h:f0 + P].rearrange(
                                  "p a b -> p (a b)"))
                      else:
                          # tiny f-tile: also load a packed [68,500] view so
                          # its stats cost ~1/4 of the free-size-bound ops
                          Ap = ap_.tile([4 * P, 500], F32, tag="a2p",
                                        name="A2p")
                          QS[0].dma_start(
                              A[:], x_d[s, f0:f0 + P].rearrange(
                                  "p a b -> p (a b)"))
                          QS[1].dma_start(
                              Ap[:], x_d[s, f0:f0 + P].rearrange(
                                  "p (k c) b -> (p k) (c b)", k=4))
                          As[2] = Ap
                      As[3 if g == 2 else g] = A

            def emit_stats(s):
                  mom = psp.tile([1, 2 * NB], F32, tag="bank6", name="mom")
                  stats = []
                  As = aload[s]
                  pstate[s] = (mom, As)
                  for g, (f0, P) in enumerate(FT):
                      if g == 2:
                          A = As[2]       # packed [68, 500] view (A2p)
                          P = 4 * P
                          rhs31 = ind2p
                      else:
                          A = As[g]
                          rhs31 = ind_sb[g]
                      stat = smp.tile([P, 2], F32, tag="stat")
                      stats.append(stat)
                      cols = 500 if g == 2 else 2000
                      Asq = sqp.tile([P, cols], F32, tag="sq",
                                     name="Asq")
                      nc.scalar.activation(Asq[:], A[:], ActFn.Square,
                                           accum_out=stat[:, 1:2])
                      nc.vector.tensor_reduce(stat[:, 0:1], A[:],
                                              mybir.AxisListType.X,
                                              AluOp.add)
                      b0, b1 = FT_BANDS[g]
                      nc.tensor.matmul(mom[0:1, b0:b1], lhsT=stat[:, 0:1],
                                       rhs=rhs31[:, b0:b1],
                                       start=True, stop=True)
                      nc.tensor.matmul(mom[0:1, NB + b0:NB + b1],
                                       lhsT=stat[:, 1:2],
                                       rhs=rhs31[:, b0:b1],
                                       start=True, stop=True)

            def emit_post(s):
                  mom, As = pstate[s]
                  # moments -> s, -mu*s on partition 0
                  m2 = smp.tile([1, 2 * NB], F32, tag="m2")
                  nc.vector.tensor_tensor(m2[:], mom[:], invct_sb[:],
                                          AluOp.mult)   # [mu | ex2]
                  mu = m2[:, 0:NB]
                  var = smp.tile([1, NB], F32, tag="var")
                  nc.vector.tensor_tensor(var[:], mu, mu, AluOp.mult)
                  nc.vector.tensor_tensor(var[:], m2[:, NB:2 * NB], var[:],
                                          AluOp.subtract)
                  sd = smp.tile([1, NB], F32, tag="sd")
                  nc.scalar.activation(sd[:], var[:], ActFn.Sqrt,
                                       bias=epsc[:])
                  vrow = smp.tile([1, 64], F32, tag="vrow")
                  nc.vector.reciprocal(vrow[:, 0:NB], sd[:])         # s
                  tmp = smp.tile([1, NB], F32, tag="tmp")
                  nc.vector.tensor_tensor(tmp[:], mu, vrow[:, 0:NB],
                                          AluOp.mult)
                  nc.vector.tensor_scalar(vrow[:, NB:2 * NB], tmp[:], -1.0,
                                          None, AluOp.mult)          # -mu*s
                  nc.vector.memset(vrow[:, 62:63], 1.0)

                  v63p = psp.tile([63, 1], F32, tag="bank7", name="v63p")
                  nc.tensor.transpose(v63p[:], vrow[:, 0:63], ident[:])
                  nc.vector.tensor_copy(v63[s][:], v63p[:])
                  nc.vector.tensor_copy(v63f[s][:], v63p[:])

                  # per-f-row s scale (s_frow = indT^T @ s)
                  for g, (f0, P) in enumerate(FT):
                      sfp = psp.tile([P, 1], F32, tag=f"bank{6 + g % 2}",
                                     name=f"sfp{g}")
                      nc.tensor.matmul(sfp[:], lhsT=indT_sb[g][:],
                                       rhs=v63[s][0:NB, :],
                                       start=True, stop=True)
                      nc.vector.tensor_copy(sfrow[s][g][:], sfp[:])

                  # bias values (bank-order cols): row matmuls for the spare-
                  # row tiles, a full [128,512] block for group 6 (tile 4)
                  nc.vector.tensor_scalar(v63rep[s][:], ones63[:],
                                          v63f[s][:, 0:1], None, AluOp.mult)
                  for j in range(8):
                      if j == 6:
                          bps = psp.tile([128, 512], F32, tag="bank6",
                                         name=f"bps{j}")
                          nc.tensor.matmul(
                              bps[:], lhsT=v63rep[s][:],
                              rhs=mbias_sb[:, j * 512:(j + 1) * 512],
                              start=True, stop=True)
                          nc.vector.tensor_copy(bblk[s][:], bps[:])
                      else:
                          bps = psp.tile([1, 512], F32, tag=f"bank{j % 2 + 6}",
                                         name=f"bps{j}")
                          nc.tensor.matmul(
                              bps[:], lhsT=v63[s][:],
                              rhs=mbias_sb[:, j * 512:(j + 1) * 512],
                              start=True, stop=True)
                          dst = brow[s][0:1, j * 512:(j + 1) * 512]
                          if j % 2 == 0:
                              nc.vector.tensor_copy(dst, bps[:])
                          else:
                              nc.scalar.copy(dst, bps[:])

                  # write the per-sample bias rows into wt_all spare rows
                  for t in range(N_XT):
                      if BIAS_ROW[t] is None:
                          continue
                      glist = [g for g in range(8) if GROUPS[g][0] == t]
                      c0 = WTOFF[glist[0]]
                      c1 = WTOFF[glist[-1]] + GCOLS[glist[-1]]
                      QS[t % 2].dma_start(
                          wt_all[TILE_ROWS[t] - 1:TILE_ROWS[t], c0:c1],
                          brow[s][0:1, c0:c1])

                  # de-interleave with s scaling, then slab DMAs
                  for g, (f0, P) in enumerate(FT):
                      Anat = As[g] if g < 2 else As[3]
                      Av = Anat[:].rearrange("p (t r) -> p r t", r=2)
                      E = eop.tile([P, T], F16, tag="eo", name="E")
                      O = eop.tile([P, T], F16, tag="eo", name="O")
                      nc.vector.tensor_scalar(E[:], Av[:, 0, :],
                                              sfrow[s][g][:, 0:1], None,
                                              AluOp.mult)
                      nc.scalar.activation(O[:], Av[:, 1, :], ActFn.Copy,
                                           scale=sfrow[s][g][:, 0:1])
                      for t in range(N_XT):
                          e = ESUM[t]
                          for (gg, srow, drow, rows) in SLABS[t]:
                              if gg != g:
                                  continue
                              q1 = QS[t % 2]
                              q2 = QS[(t + 1) % 2]
                              q1.dma_start(xg[s][t][drow:drow + rows, :],
                                           E[srow:srow + rows, :])
                              q2.dma_start(
                                  xg[s][t][e + drow:e + drow + rows, :],
                                  O[srow:srow + rows, :])

            # ---------------- main loop ----------------
            def emit_chunk(s, t0, M, ci=0):
                      ob = outp.tile([128, CH * NB], F16, tag="ob")
                      ob_v = ob[0:M].rearrange("p (o i) -> p o i", o=CH, i=NB)
                      pview = {j: psp.tile([128, 512], F32, tag=f"bank{j}",
                                           name=f"ps{j}") for j in range(8)}
                      for gi, (t, blo, bhi, bank) in enumerate(GROUPS):
                          n = GCOLS[gi]
                          rend = GREND[gi]
                          nc.tensor.matmul(
                              pview[bank][0:M, 0:n],
                              lhsT=xg[s][t][0:rend, t0:t0 + M],
                              rhs=wt_all[0:rend, WTOFF[gi]:WTOFF[gi] + n],
                              start=True, stop=True)
                      # drains
                      for gi, (t, blo, bhi, bank) in enumerate(GROUPS):
                          nb = bhi - blo
                          n = GCOLS[gi]
                          dst = ob_v[:, :, blo:bhi]
                          src = pview[bank][0:M, 0:n].rearrange(
                              "p (o i) -> p o i", o=CH, i=nb)
                          eng = DRAIN_ENG[gi]
                          if eng == "s":
                              nc.scalar.copy(dst, src)
                          elif eng == "v":
                              nc.vector.tensor_copy(dst, src)
                          else:  # "vb": vector copy + bias add (tile 4)
                              bia = bblk[s][0:M, 0:n].rearrange(
                                  "p (o i) -> p o i", o=CH, i=nb)
                              nc.vector.tensor_tensor(dst, src, bia,
                                                      AluOp.add)
                      zq = [nc.sync, nc.gpsimd, nc.scalar][ci % 3]
                      zq.dma_start(z_d[s, t0:t0 + M], ob[0:M, :])

            # Emission order matters: engine queues are in-order.  A-loads
            # go first (consts behind them), s1's prologue PE ops slot
            # between early s0 chunks.
            for _rep in range(repeat):
                emit_loads(0)
                if _rep == 0:
                    emit_consts()
                    emit_setup()
                emit_stats(0)
                emit_post(0)
                emit_loads(1)
                ci = 0
                for (t0, M) in CHUNKS[:3]:
                    emit_chunk(0, t0, M, ci); ci += 1
                emit_stats(1)
                for (t0, M) in CHUNKS[3:5]:
                    emit_chunk(0, t0, M, ci); ci += 1
                emit_post(1)
                for (t0, M) in CHUNKS[5:]:
                    emit_chunk(0, t0, M, ci); ci += 1
                for (t0, M) in CHUNKS:
                    emit_chunk(1, t0, M, ci); ci += 1

    _NC_CACHE[key] = nc
    return nc


# ----------------------------------------------------------------------------
# Public entry point
# ----------------------------------------------------------------------------
def kernel(x, gn_w, gn_b, fc_w, fc_b):
    x = np.asarray(x, np.float32)
    gn_w = np.asarray(gn_w, np.float32)
    gn_b = np.asarray(gn_b, np.float32)
    fc_w = np.asarray(fc_w, np.float32)
    fc_b = np.asarray(fc_b, np.float32)

    (wt_all, mbias, indT, invct2, ind128,
     ones1000) = _build_const_tables(gn_w, gn_b, fc_w, fc_b)
    nc = build_bass()
    if not getattr(nc, "_waits_spilled", False):
        _spill_waits(nc)
        nc._waits_spilled = True

    in_maps = []
    for k in range(N_CORES):
        m = {"x": np.ascontiguousarray(x[k * B_LOC:(k + 1) * B_LOC]),
             "wt": wt_all, "mbias": mbias, "indT": indT, "invct2": invct2,
             "ind128": ind128, "ones1000": ones1000}
        in_maps.append(m)
    res = run_bass_kernel_spmd(nc, in_maps, core_ids=list(range(N_CORES)))
    z16 = np.concatenate([r["z"] for r in res.results], axis=0)
    return z16.reshape(B_FULL, T, CH, NB).astype(np.float32)
